# revision 1
# baseline (speedup 1.0000x reference)
"""ChebConv (complex, K+1=3 hops) Trainium2 kernel over 8 NeuronCores.

Sharding: 1D node partition on destination rows (6250 rows/core), full X
replicated; each core processes exactly the edges targeting its rows, so no
collectives are needed.

Per core the computation is reorganized as, for each 21-row "group" g and
each 128-edge block b (edges sorted by destination group, split by col half
for int16 gather indices):

  G   = Xcat[cols[block]]                  # dma_gather, [128 edges, 512] bf16
  V   = onehot(jlocal) * C6                # [128 edges, 6*21]  (DVE)
  P_q += G[:, q*128:(q+1)*128].T @ V       # PE, PSUM [128 feat, 126] x4

P holds all four spmm partial aggregates for the group, transposed
(features on partitions).  Per batch of 6 groups, 24 more PE matmuls with
signed weight tiles contract features and produce the row-major outputs
(real/imag) directly; bias is added during the PSUM->SBUF copy.
"""
import sys
sys.path.insert(0, '/opt/trn_rl_repo')

import numpy as np
import ml_dtypes

N = 50000
E = 1_600_000
K1 = 3
C = 256
CORES = 8
RPC = N // CORES            # 6250 rows per core
GR = 21                     # rows per group
MCOLS = 6 * GR              # 126 one-hot columns
GPB = 6                     # groups per batch
ROWS_PB = GR * GPB          # 126
NB = -(-RPC // ROWS_PB)     # 50
NGRP = NB * GPB             # 300
REAL_GRP = -(-RPC // GR)    # 298
HALF = 32768
NQ = 4                      # SWDGE queues


def _bf16(x):
    return x.astype(ml_dtypes.bfloat16)


def _preprocess(rows, cols, Lr, Li, weight, bias):
    rows = np.asarray(rows).astype(np.int64)
    cols = np.asarray(cols).astype(np.int64)
    core = rows // RPC
    rloc = rows - core * RPC
    g = rloc // GR
    jl = (rloc - g * GR).astype(np.float32)

    C6 = np.empty((E, 6), np.float32)
    C6[:, 0:3] = np.asarray(Lr).T
    C6[:, 3:6] = np.asarray(Li).T

    # order all edges by (core, group, col-half); stable so layout is
    # deterministic
    colh = (cols >= HALF).astype(np.int64)
    key = ((core * NGRP) + g) * 2 + colh
    order = np.argsort(key, kind="stable")
    key_s = key[order]
    # bucket boundaries over core*group*half
    nbuck = CORES * NGRP * 2
    bounds = np.searchsorted(key_s, np.arange(nbuck + 1))
    cnt = (bounds[1:] - bounds[:-1]).reshape(CORES, NGRP, 2)

    # shared block counts per (group, half) = max over cores
    nblk_h = -(-cnt.max(axis=0) // 128)          # [NGRP, 2]
    empty = nblk_h.sum(axis=1) == 0
    nblk_h[:REAL_GRP, 0] = np.maximum(nblk_h[:REAL_GRP, 0], 1)
    tot_blk = int(nblk_h.sum())

    # per-block metadata (shared): group, half, and call boundaries
    blk_g = np.empty(tot_blk, np.int64)
    blk_off_h = np.zeros((NGRP, 2), np.int64)    # starting block of (g,h)
    b0 = 0
    calls = []                                    # (g, h, blk_start, nblk)
    for gi in range(NGRP):
        for h in range(2):
            nb = int(nblk_h[gi, h])
            if nb == 0:
                continue
            blk_off_h[gi, h] = b0
            blk_g[b0:b0 + nb] = gi
            calls.append((gi, h, b0, nb))
            b0 += nb
    assert b0 == tot_blk

    # per-core arrays
    per_core = []
    cols_s = cols[order]
    C6_s = C6[order]
    jl_s = jl[order]
    for c in range(CORES):
        idx16 = np.zeros(tot_blk * 128, np.int16)
        c6t = np.zeros((128, tot_blk * 6), np.float32)
        jlf = np.zeros((128, tot_blk), np.float32)
        for gi, h, bs, nb in calls:
            buck = (c * NGRP + gi) * 2 + h
            lo, hi = bounds[buck], bounds[buck + 1]
            ne = hi - lo
            if ne == 0:
                continue
            sl = slice(bs * 128, bs * 128 + ne)
            idx16[sl] = (cols_s[lo:hi] - h * HALF).astype(np.int16)
            cc = C6_s[lo:hi]
            jj = jl_s[lo:hi]
            for k in range(nb):
                a, b = k * 128, min((k + 1) * 128, ne)
                if a >= b:
                    break
                c6t[0:b - a, (bs + k) * 6:(bs + k) * 6 + 6] = cc[a:b]
                jlf[0:b - a, bs + k] = jj[a:b]
        # wrap idxs: idx i lives at [i%16, i//16]; replicate to 128 partitions
        idxw = np.tile(idx16.reshape(-1, 16).T, (8, 1))  # [128, tot_blk*8]
        per_core.append(dict(
            idx=np.ascontiguousarray(idxw),
            c6=np.ascontiguousarray(_bf16(c6t)),
            jl=np.ascontiguousarray(jlf),
        ))

    # weight tiles [12][128, 256] f32: 0..5 = +W[k][fh], 6..11 = -W[k][fh]
    weight = np.asarray(weight, np.float32)
    wt = np.empty((12, 128, C), np.float32)
    for fh in range(2):
        for k in range(K1):
            wt[fh * 3 + k] = weight[k][fh * 128:(fh + 1) * 128]
            wt[6 + fh * 3 + k] = -weight[k][fh * 128:(fh + 1) * 128]
    wsb = np.ascontiguousarray(wt.transpose(1, 0, 2).reshape(128, 12 * C))

    biasr = np.ascontiguousarray(np.tile(np.asarray(bias, np.float32), (128, 1)))
    # V column layout is plane-major: m = s*21 + j  ->  j = m % 21
    mdiv6 = np.ascontiguousarray(
        _bf16(np.tile((np.arange(MCOLS) % GR).astype(np.float32), (128, 1))))

    return dict(nblk_h=nblk_h, tot_blk=tot_blk, blk_g=blk_g, calls=calls,
                empty=empty, per_core=per_core, wsb=wsb, biasr=biasr,
                mdiv6=mdiv6)


def _final_mm_list():
    """(target, q, s, wtile): target 0=real 1=imag; q = P region; s = slot."""
    mms = []
    for tgt in range(2):
        for fh in range(2):
            for k in range(K1):
                if tgt == 0:
                    mms.append((0, fh, k, fh * 3 + k))           # +W  P_r
                    mms.append((0, 2 + fh, 3 + k, 6 + fh * 3 + k))  # -W P_i
                else:
                    mms.append((1, fh, 3 + k, fh * 3 + k))       # +W  P_r
                    mms.append((1, 2 + fh, k, fh * 3 + k))       # +W  P_i
    return mms


def _build(nc, prep, repeat=1):
    import concourse.mybir as mybir
    from concourse.tile import TileContext

    f32 = mybir.dt.float32
    bf16 = mybir.dt.bfloat16
    i16 = mybir.dt.int16
    tot_blk = prep["tot_blk"]
    nblk_h = prep["nblk_h"]
    calls = prep["calls"]
    blk_g = prep["blk_g"]
    empty = prep["empty"]

    xcat = nc.dram_tensor("xcat", [N, 512], bf16, kind="ExternalInput")
    idx_d = nc.dram_tensor("idx", [128, tot_blk * 8], i16, kind="ExternalInput")
    c6_d = nc.dram_tensor("c6", [128, tot_blk * 6], bf16, kind="ExternalInput")
    jl_d = nc.dram_tensor("jl", [128, tot_blk], f32, kind="ExternalInput")
    w_d = nc.dram_tensor("wt", [128, 12 * C], f32, kind="ExternalInput")
    bias_d = nc.dram_tensor("biasr", [128, C], f32, kind="ExternalInput")
    md_d = nc.dram_tensor("mdiv6", [128, MCOLS], bf16, kind="ExternalInput")
    or_d = nc.dram_tensor("out_r", [NB * ROWS_PB, C], f32, kind="ExternalOutput")
    oi_d = nc.dram_tensor("out_i", [NB * ROWS_PB, C], f32, kind="ExternalOutput")

    mms = _final_mm_list()

    import contextlib

    with TileContext(nc) as tc:
        with tc.tile_pool(name="const", bufs=1) as cpool, \
             tc.tile_pool(name="g", bufs=6) as gpool, \
             tc.tile_pool(name="v", bufs=28) as vpool, \
             tc.tile_pool(name="pb", bufs=2) as pbpool, \
             tc.tile_pool(name="os", bufs=4) as ospool, \
             tc.tile_pool(name="ps", bufs=2, space="PSUM") as pspool, \
             tc.tile_pool(name="po", bufs=2, space="PSUM") as popool:

            idx_t = cpool.tile([128, tot_blk * 8], i16)
            c6_t = cpool.tile([128, tot_blk * 6], bf16)
            jl_t = cpool.tile([128, tot_blk], f32)
            w_t = cpool.tile([128, 12 * C], f32)
            bias_t = cpool.tile([128, C], f32)
            md_t = cpool.tile([128, MCOLS], bf16)
            for dst, src in [(idx_t, idx_d), (c6_t, c6_d), (jl_t, jl_d),
                             (w_t, w_d), (bias_t, bias_d), (md_t, md_d)]:
                nc.sync.dma_start(dst[:], src[:])

            rep_cm = tc.For_i(0, repeat, 1) if repeat > 1 else contextlib.nullcontext()
            with rep_cm:
              qn = 0
              call_i = 0
              for bt in range(NB):
                  pbuf = pbpool.tile([128, GPB * 504], f32, tag="pbuf")
                  for gl in range(GPB):
                      gi = bt * GPB + gl
                      nb_tot = int(nblk_h[gi].sum())
                      if nb_tot == 0:
                          nc.vector.memset(
                              pbuf[:].rearrange(
                                  "p (q s g j) -> p q s g j", q=4, s=6, g=GPB)[
                                  :, :, :, gl, :], 0.0)
                          continue
                      gt = gpool.tile([128, nb_tot * 512], bf16, tag="g")
                      # gather calls for this group (up to 2: col halves)
                      done = 0
                      while call_i < len(calls) and calls[call_i][0] == gi:
                          _, h, bs, nb = calls[call_i]
                          nidx = nb * 128
                          src = xcat[:] if h == 0 else xcat[HALF:, :]
                          nc.gpsimd.dma_gather(
                              gt[:, done * 512:(done + nb) * 512]
                                .rearrange("p (b e) -> p b e", e=512),
                              src,
                              idx_t[:, bs * 8:(bs + nb) * 8],
                              nidx, nidx, 512,
                              queue_num=qn,
                          )
                          qn = (qn + 1) % NQ
                          done += nb
                          call_i += 1
                      assert done == nb_tot
                      bs0 = int(nblk_h[:gi].sum()) if gi else 0
                      p_t = pspool.tile([128, 504], f32, tag="p")
                      v_ts = []
                      for b in range(nb_tot):
                          gb = bs0 + b
                          assert blk_g[gb] == gi
                          v_t = vpool.tile([128, MCOLS], bf16, tag="v")
                          nc.vector.tensor_scalar(
                              v_t[:], md_t[:], jl_t[:, gb:gb + 1], None,
                              mybir.AluOpType.is_equal)
                          c6rep = c6_t[:, gb * 6:gb * 6 + 6] \
                              .unsqueeze(2).broadcast_to((128, 6, GR))
                          nc.vector.tensor_tensor(
                              v_t[:].rearrange("p (s x) -> p s x", x=GR),
                              v_t[:].rearrange("p (s x) -> p s x", x=GR),
                              c6rep, mybir.AluOpType.mult)
                          v_ts.append(v_t)
                      for q in range(4):
                          for b in range(nb_tot):
                              nc.tensor.matmul(
                                  p_t[:, q * 126:(q + 1) * 126],
                                  gt[:, b * 512 + q * 128:b * 512 + (q + 1) * 128],
                                  v_ts[b][:],
                                  start=(b == 0), stop=(b == nb_tot - 1))
                      # pbuf is plane-major over the whole batch: column
                      # (q*6+s)*126 + 21*gl + j; p_t columns are q*126+s*21+j
                      pb_dst = pbuf[:].rearrange(
                          "p (q s g j) -> p q s g j", q=4, s=6, g=GPB)[
                          :, :, :, gl, :]
                      nc.scalar.copy(pb_dst, p_t[:])
                  # final matmuls for this batch
                  po_r = popool.tile([128, C], f32, tag="por")
                  po_i = popool.tile([128, C], f32, tag="poi")
                  nmm = {0: 0, 1: 0}
                  for tgt, q, s, wi in mms:
                      po = po_r if tgt == 0 else po_i
                      plane = q * 6 + s
                      lhsT = pbuf[:, plane * MCOLS:(plane + 1) * MCOLS]
                      nc.tensor.matmul(
                          po[:MCOLS, :], lhsT, w_t[:, wi * C:(wi + 1) * C],
                          start=(nmm[tgt] == 0), stop=(nmm[tgt] == 11))
                      nmm[tgt] += 1
                  o_r = ospool.tile([128, C], f32, tag="or")
                  o_i = ospool.tile([128, C], f32, tag="oi")
                  nc.vector.tensor_tensor(o_r[:MCOLS, :], po_r[:MCOLS, :],
                                          bias_t[:MCOLS, :], mybir.AluOpType.add)
                  nc.vector.tensor_tensor(o_i[:MCOLS, :], po_i[:MCOLS, :],
                                          bias_t[:MCOLS, :], mybir.AluOpType.add)
                  nc.sync.dma_start(or_d[bt * ROWS_PB:(bt + 1) * ROWS_PB, :],
                                    o_r[:MCOLS, :])
                  nc.sync.dma_start(oi_d[bt * ROWS_PB:(bt + 1) * ROWS_PB, :],
                                    o_i[:MCOLS, :])
              assert call_i == len(calls)


def _make_nc(prep, repeat=1):
    import concourse.bacc as bacc
    nc = bacc.Bacc("TRN2", target_bir_lowering=False, debug=False,
                   num_swdge_queues=NQ)
    _build(nc, prep, repeat=repeat)
    nc.compile()
    return nc


def kernel(X_real, X_imag, L_real_vals, L_imag_vals, weight, bias, rows, cols):
    from concourse.bass_utils import run_bass_kernel_spmd

    prep = _preprocess(rows, cols, L_real_vals, L_imag_vals, weight, bias)
    nc = _make_nc(prep)

    xcat = _bf16(np.concatenate(
        [np.asarray(X_real, np.float32), np.asarray(X_imag, np.float32)], axis=1))
    in_maps = []
    for c in range(CORES):
        pc = prep["per_core"][c]
        in_maps.append({
            "xcat": xcat, "idx": pc["idx"], "c6": pc["c6"], "jl": pc["jl"],
            "wt": prep["wsb"], "biasr": prep["biasr"], "mdiv6": prep["mdiv6"],
        })
    res = run_bass_kernel_spmd(nc, in_maps, core_ids=list(range(CORES)))
    out_r = np.concatenate([res.results[c]["out_r"][:RPC] for c in range(CORES)], 0)
    out_i = np.concatenate([res.results[c]["out_i"][:RPC] for c in range(CORES)], 0)
    return out_r, out_i

